# revision 10
# baseline (speedup 1.0000x reference)
"""KNN k-th-nearest-distance kernel for Trainium2 (8 NeuronCores).

Math: reference computes, per query row n, the k-th smallest of
dist[n,m] = sqrt(max(||zn||^2 + ||rn||^2 - 2 zn.rn, 1e-12)) over m,
with zn/rn the L2-normalized rows of z [2048,128] / ref [50000,128].
Since ||zn||^2 == ||rn||^2 == 1 (to fp32 rounding), dist is a
monotonically *decreasing* function of s = zn.rn, so the k-th smallest
distance corresponds to the k-th *largest* inner product s.

Device work, v4 (ref sharded across cores, queries replicated):
  Phase A (per core, own 1/8 ref slice, zero-padded to 6400 rows):
    normalize rows in fp32 (square-accum on ScalarE, reciprocal on
    VectorE, scale on GpSimd), PE-transpose -> rnT slice [128, 6400]
    fp32 resident in SBUF. 1/norm = 1/max(sqrt(ssq), 1e-20) so padded
    zero rows give sims of exactly 0 (never competitive: real top-11
    sims are ~ +0.37|z| for randn data).
  Phase B: for each of the 16 query blocks, s_raw = z_blk @ rnT_slice
    via fp32 PE matmul into 2048-wide PSUM windows; DVE max8 per
    window -> per-(block, window) top-8 candidates.
  Merge: per block, max8 + match_replace + max8 -> this core's local
    top-16 per query; AllGather the [2048,16] candidate matrices
    (128KB) and reduce 8x16 -> exact global top-16 per query.
Host: divide by |z_n|, dist = sqrt(max(2 - 2*s, 1e-12)), pick column k.
"""

import os
import sys

sys.path.insert(0, "/opt/trn_rl_repo")

from contextlib import ExitStack

import numpy as np


def _install_ntff_hook_shim():
    """The agent image's antenv lacks axon_hooks, so trace=True degrades.
    Recreate the hook module + the ctypes NTFF driver (mirrors
    trn_agent_boot.trn_boot._ntff_profile_via_ctypes)."""
    import contextlib
    import ctypes
    import types

    if "antenv.axon_hooks" in sys.modules:
        return
    so_path = "/opt/axon/libaxon_pjrt.so"
    mod = types.ModuleType("antenv.axon_hooks")
    state = {"hook": None}

    def set_axon_ntff_profile_hook(h):
        state["hook"] = h

    def get_axon_ntff_profile_hook():
        return state["hook"]

    mod.set_axon_ntff_profile_hook = set_axon_ntff_profile_hook
    mod.get_axon_ntff_profile_hook = get_axon_ntff_profile_hook
    sys.modules["antenv.axon_hooks"] = mod

    try:
        lib = ctypes.CDLL(so_path)
        if not hasattr(lib, "axon_start_nrt_profile"):
            return
        lib.axon_start_nrt_profile.argtypes = [
            ctypes.POINTER(ctypes.c_int64),
            ctypes.c_size_t,
        ]
        lib.axon_start_nrt_profile.restype = ctypes.c_int64
        lib.axon_stop_nrt_profile.argtypes = [ctypes.c_char_p]
        lib.axon_stop_nrt_profile.restype = ctypes.c_int64

        @contextlib.contextmanager
        def _hook(output_dir, device_ids):
            import jax

            jax.devices()
            if device_ids:
                ids = (ctypes.c_int64 * len(device_ids))(*device_ids)
                rc = lib.axon_start_nrt_profile(ids, len(device_ids))
            else:
                rc = lib.axon_start_nrt_profile(None, 0)
            if rc != 0:
                raise RuntimeError(f"axon_start_nrt_profile rc={rc}")
            try:
                yield
            finally:
                n = lib.axon_stop_nrt_profile(str(output_dir).encode())
                print(f"ntff profile: {n} file(s) written to {output_dir}")

        state["hook"] = _hook
    except Exception:
        pass


_install_ntff_hook_shim()

import concourse.bacc as bacc
import concourse.bass as bass
import concourse.tile as tile
from concourse import mybir
from concourse.bass_utils import run_bass_kernel_spmd

N, M, D = 2048, 50000, 128
NCORES = 8
NBLK = N // 128  # 16 query blocks of 128
SLICE = 6400  # ref rows prepped per core (core 7 zero-padded)
WINS = (2048, 2048, 2048, 256)  # max8 windows over the 6400 slice cols
NWIN = len(WINS)
CANDW = NWIN * 8  # per-core candidate width per query row (32)

F32 = mybir.dt.float32

_CACHE = {}
LAST_RESULTS = None


def _build():
    nc = bacc.Bacc(
        "TRN2", target_bir_lowering=False, debug=False, num_devices=NCORES
    )
    z_d = nc.dram_tensor("z", [N, D], F32, kind="ExternalInput")
    ref_d = nc.dram_tensor("refsl", [SLICE, D], F32, kind="ExternalInput")
    out_d = nc.dram_tensor("top16", [N, 16], F32, kind="ExternalOutput")
    ident_d = nc.inline_tensor(np.eye(128, dtype=np.float32), name="ident")

    z = z_d.ap()
    ref = ref_d.ap()
    out16 = out_d.ap()

    with tile.TileContext(nc) as tc, ExitStack() as ctx:
        const_pool = ctx.enter_context(tc.tile_pool(name="const", bufs=1))
        persist = ctx.enter_context(tc.tile_pool(name="persist", bufs=1))
        cand_pool = ctx.enter_context(tc.tile_pool(name="cand", bufs=1))
        dram = ctx.enter_context(tc.tile_pool(name="dram", bufs=1, space="DRAM"))
        fin_pool = ctx.enter_context(tc.tile_pool(name="fin", bufs=2))

        ident = const_pool.tile([128, 128], F32, name="ident")
        nc.sync.dma_start(ident[:, :], ident_d.ap()[:, :])

        rnT_sl = persist.tile([128, SLICE], F32, name="rnT_sl")
        znT = [
            persist.tile([128, 128], F32, tag=f"znT{b}", name=f"znT{b}")
            for b in range(NBLK)
        ]
        cand = [
            cand_pool.tile([128, CANDW], F32, tag=f"cand{b}", name=f"cand{b}")
            for b in range(NBLK)
        ]
        lg = dram.tile([N, 16], F32, name="lg")
        lgall = dram.tile([NCORES, N, 16], F32, name="lgall")

        # ---- Phase A: z transposes + own-slice normalize/transpose ----
        with (
            tc.tile_pool(name="rload", bufs=3) as rload_pool,
            tc.tile_pool(name="sq", bufs=2) as sq_pool,
            tc.tile_pool(name="stat", bufs=3) as stat_pool,
            tc.tile_pool(name="rsc", bufs=3) as rsc_pool,
            tc.tile_pool(name="tps", bufs=2, space="PSUM") as tpsum_pool,
            tc.tile_pool(name="zl", bufs=3) as zl_pool,
        ):
            for b in range(NBLK):
                zt = zl_pool.tile([128, D], F32, tag="zload")
                nc.sync.dma_start(zt[:, :], z[b * 128 : (b + 1) * 128, :])
                zp = tpsum_pool.tile([128, 512], F32, tag="tps")
                nc.tensor.transpose(zp[:, :128], zt[:, :], ident[:, :])
                nc.scalar.copy(znT[b][:, :], zp[:, :128])

            ngrp = SLICE // 512  # 12 full groups + one 256 tail
            grps = [512] * ngrp + ([SLICE % 512] if SLICE % 512 else [])
            col = 0
            for gw in grps:
                G = gw // 128
                rl = rload_pool.tile([128, 4, 128], F32, tag="rload")
                nc.sync.dma_start(
                    rl[:, :G, :],
                    ref[col : col + gw, :].rearrange("(g p) d -> p g d", p=128),
                )
                ssq = stat_pool.tile([128, 4], F32, tag="ssq")
                for g in range(G):
                    sq = sq_pool.tile([128, 128], F32, tag="sq")
                    nc.scalar.activation(
                        sq[:, :],
                        rl[:, g, :],
                        mybir.ActivationFunctionType.Square,
                        accum_out=ssq[:, g : g + 1],
                    )
                sn = stat_pool.tile([128, 4], F32, tag="sn")
                nc.scalar.sqrt(sn[:, :G], ssq[:, :G])
                # clamp so zero-padded rows give 0*big = 0, not NaN
                sc = stat_pool.tile([128, 4], F32, tag="sc")
                nc.vector.tensor_scalar_max(sc[:, :G], sn[:, :G], 1e-20)
                rq = stat_pool.tile([128, 4], F32, tag="rq")
                nc.vector.reciprocal(rq[:, :G], sc[:, :G])

                rsc = rsc_pool.tile([128, 4, 128], F32, tag="rsc")
                for g in range(G):
                    nc.gpsimd.tensor_scalar_mul(
                        rsc[:, g, :], rl[:, g, :], rq[:, g : g + 1]
                    )
                tp = tpsum_pool.tile([128, 512], F32, tag="tps")
                for g in range(G):
                    nc.tensor.transpose(
                        tp[:, g * 128 : (g + 1) * 128],
                        rsc[:, g, :],
                        ident[:, :],
                    )
                nc.scalar.copy(rnT_sl[:, col : col + gw], tp[:, :gw])
                col += gw

        # ---- Phase B: per query block, matmul over slice + max8 ----
        with tc.tile_pool(name="sp", bufs=2, space="PSUM") as spsum_pool:
            for b in range(NBLK):
                pcol = 0
                for w_i, w in enumerate(WINS):
                    sp = spsum_pool.tile([128, 2048], F32, tag="sp")
                    for h in range(0, w, 512):
                        hw = min(512, w - h)
                        nc.tensor.matmul(
                            sp[:, h : h + hw],
                            znT[b][:, :],
                            rnT_sl[:, pcol + h : pcol + h + hw],
                            start=True,
                            stop=True,
                        )
                    nc.vector.max(
                        cand[b][:, w_i * 8 : (w_i + 1) * 8], sp[:, :w]
                    )
                    pcol += w

        # ---- local merge: per block top-16 of the 32 candidates ----
        for b in range(NBLK):
            t8a = fin_pool.tile([128, 8], F32, tag="t8a")
            nc.vector.max(t8a[:, :], cand[b][:, :])
            cand2 = fin_pool.tile([128, CANDW], F32, tag="cand2")
            nc.vector.match_replace(cand2[:, :], t8a[:, :], cand[b][:, :], -3.0)
            t8b = fin_pool.tile([128, 8], F32, tag="t8b")
            nc.vector.max(t8b[:, :], cand2[:, :])
            nc.sync.dma_start(lg[b * 128 : (b + 1) * 128, 0:8], t8a[:, :])
            nc.sync.dma_start(lg[b * 128 : (b + 1) * 128, 8:16], t8b[:, :])

        # ---- global merge: gather 8x16 candidates, reduce to top-16 ----
        nc.gpsimd.collective_compute(
            "AllGather",
            mybir.AluOpType.bypass,
            replica_groups=[list(range(NCORES))],
            ins=[lg.opt()],
            outs=[lgall.opt()],
        )
        for b in range(NBLK):
            gc = fin_pool.tile([128, NCORES, 16], F32, tag="gc")
            nc.sync.dma_start(
                gc[:, :, :],
                lgall[:, b * 128 : (b + 1) * 128, :].rearrange(
                    "c p j -> p c j"
                ),
            )
            g8a = fin_pool.tile([128, 8], F32, tag="g8a")
            nc.vector.max(g8a[:, :], gc[:, :, :])
            gc2 = fin_pool.tile([128, NCORES * 16], F32, tag="gc2")
            nc.vector.match_replace(
                gc2[:, :], g8a[:, :], gc[:, :, :], -3.0
            )
            g8b = fin_pool.tile([128, 8], F32, tag="g8b")
            nc.vector.max(g8b[:, :], gc2[:, :])
            nc.sync.dma_start(out16[b * 128 : (b + 1) * 128, 0:8], g8a[:, :])
            nc.sync.dma_start(out16[b * 128 : (b + 1) * 128, 8:16], g8b[:, :])

    nc.compile()
    return nc


def kernel(z, ref, k):
    global LAST_RESULTS
    z_np = np.ascontiguousarray(np.asarray(z, dtype=np.float32))
    ref_np = np.ascontiguousarray(np.asarray(ref, dtype=np.float32))
    kk = int(k)

    if "nc" not in _CACHE:
        _CACHE["nc"] = _build()
    nc = _CACHE["nc"]

    refp = np.zeros((NCORES * SLICE, D), dtype=np.float32)
    refp[:M] = ref_np
    in_maps = [
        {
            "z": z_np,
            "refsl": np.ascontiguousarray(refp[i * SLICE : (i + 1) * SLICE]),
        }
        for i in range(NCORES)
    ]
    res = run_bass_kernel_spmd(nc, in_maps, core_ids=list(range(NCORES)))
    LAST_RESULTS = res
    top16 = res.results[0]["top16"]  # all cores produce the same merged [N,16]

    znorm = np.sqrt(np.sum(z_np.astype(np.float32) ** 2, axis=1))  # [N]
    s = top16[:, kk] / znorm
    return np.sqrt(np.maximum(2.0 - 2.0 * s, 1e-12)).astype(np.float32)


# revision 11
# speedup vs baseline: 1.6738x; 1.6738x over previous
"""KNN k-th-nearest-distance kernel for Trainium2 (8 NeuronCores).

Math: reference computes, per query row n, the k-th smallest of
dist[n,m] = sqrt(max(||zn||^2 + ||rn||^2 - 2 zn.rn, 1e-12)) over m,
with zn/rn the L2-normalized rows of z [2048,128] / ref [50000,128].
Since ||zn||^2 == ||rn||^2 == 1 (to fp32 rounding), dist is a
monotonically *decreasing* function of s = zn.rn, so the k-th smallest
distance corresponds to the k-th *largest* inner product s.

Device work, v4 (ref sharded across cores, queries replicated):
  Phase A (per core, own 1/8 ref slice, zero-padded to 6400 rows):
    normalize rows in fp32 (square-accum on ScalarE, reciprocal on
    VectorE, scale on GpSimd), PE-transpose -> rnT slice [128, 6400]
    fp32 resident in SBUF. 1/norm = 1/max(sqrt(ssq), 1e-20) so padded
    zero rows give sims of exactly 0 (never competitive: real top-11
    sims are ~ +0.37|z| for randn data).
  Phase B: for each of the 16 query blocks, s_raw = z_blk @ rnT_slice
    via fp32 PE matmul into 2048-wide PSUM windows; DVE max8 per
    window -> per-(block, window) top-8 candidates.
  Merge: per block, max8 + match_replace + max8 -> this core's local
    top-16 per query; AllGather the [2048,16] candidate matrices
    (128KB) and reduce 8x16 -> exact global top-16 per query.
Host: divide by |z_n|, dist = sqrt(max(2 - 2*s, 1e-12)), pick column k.
"""

import os
import sys

sys.path.insert(0, "/opt/trn_rl_repo")

from contextlib import ExitStack

import numpy as np


def _install_ntff_hook_shim():
    """The agent image's antenv lacks axon_hooks, so trace=True degrades.
    Recreate the hook module + the ctypes NTFF driver (mirrors
    trn_agent_boot.trn_boot._ntff_profile_via_ctypes)."""
    import contextlib
    import ctypes
    import types

    if "antenv.axon_hooks" in sys.modules:
        return
    so_path = "/opt/axon/libaxon_pjrt.so"
    mod = types.ModuleType("antenv.axon_hooks")
    state = {"hook": None}

    def set_axon_ntff_profile_hook(h):
        state["hook"] = h

    def get_axon_ntff_profile_hook():
        return state["hook"]

    mod.set_axon_ntff_profile_hook = set_axon_ntff_profile_hook
    mod.get_axon_ntff_profile_hook = get_axon_ntff_profile_hook
    sys.modules["antenv.axon_hooks"] = mod

    try:
        lib = ctypes.CDLL(so_path)
        if not hasattr(lib, "axon_start_nrt_profile"):
            return
        lib.axon_start_nrt_profile.argtypes = [
            ctypes.POINTER(ctypes.c_int64),
            ctypes.c_size_t,
        ]
        lib.axon_start_nrt_profile.restype = ctypes.c_int64
        lib.axon_stop_nrt_profile.argtypes = [ctypes.c_char_p]
        lib.axon_stop_nrt_profile.restype = ctypes.c_int64

        @contextlib.contextmanager
        def _hook(output_dir, device_ids):
            import jax

            jax.devices()
            if device_ids:
                ids = (ctypes.c_int64 * len(device_ids))(*device_ids)
                rc = lib.axon_start_nrt_profile(ids, len(device_ids))
            else:
                rc = lib.axon_start_nrt_profile(None, 0)
            if rc != 0:
                raise RuntimeError(f"axon_start_nrt_profile rc={rc}")
            try:
                yield
            finally:
                n = lib.axon_stop_nrt_profile(str(output_dir).encode())
                print(f"ntff profile: {n} file(s) written to {output_dir}")

        state["hook"] = _hook
    except Exception:
        pass


_install_ntff_hook_shim()

import concourse.bacc as bacc
import concourse.bass as bass
import concourse.tile as tile
from concourse import mybir
from concourse.bass_utils import run_bass_kernel_spmd

N, M, D = 2048, 50000, 128
NCORES = 8
NBLK = N // 128  # 16 query blocks of 128
SLICE = 6400  # ref rows prepped per core (core 7 zero-padded)
WINS = (2048, 2048, 2048, 256)  # max8 windows over the 6400 slice cols
NWIN = len(WINS)
CANDW = NWIN * 8  # per-core candidate width per query row (32)

F32 = mybir.dt.float32

_CACHE = {}
LAST_RESULTS = None


def _build():
    nc = bacc.Bacc(
        "TRN2", target_bir_lowering=False, debug=False, num_devices=NCORES
    )
    z_d = nc.dram_tensor("z", [N, D], F32, kind="ExternalInput")
    ref_d = nc.dram_tensor("refsl", [SLICE, D], F32, kind="ExternalInput")
    out_d = nc.dram_tensor("top16", [N, 16], F32, kind="ExternalOutput")
    ident_d = nc.inline_tensor(np.eye(128, dtype=np.float32), name="ident")

    z = z_d.ap()
    ref = ref_d.ap()
    out16 = out_d.ap()

    with tile.TileContext(nc) as tc, ExitStack() as ctx:
        const_pool = ctx.enter_context(tc.tile_pool(name="const", bufs=1))
        persist = ctx.enter_context(tc.tile_pool(name="persist", bufs=1))
        cand_pool = ctx.enter_context(tc.tile_pool(name="cand", bufs=1))
        fin_pool = ctx.enter_context(tc.tile_pool(name="fin", bufs=2))

        ident = const_pool.tile([128, 128], F32, name="ident")
        nc.sync.dma_start(ident[:, :], ident_d.ap()[:, :])

        rnT_sl = persist.tile([128, SLICE], F32, name="rnT_sl")
        znT = [
            persist.tile([128, 128], F32, tag=f"znT{b}", name=f"znT{b}")
            for b in range(NBLK)
        ]
        cand = [
            cand_pool.tile([128, CANDW], F32, tag=f"cand{b}", name=f"cand{b}")
            for b in range(NBLK)
        ]

        # ---- Phase A: z transposes + own-slice normalize/transpose ----
        with (
            tc.tile_pool(name="rload", bufs=3) as rload_pool,
            tc.tile_pool(name="sq", bufs=2) as sq_pool,
            tc.tile_pool(name="stat", bufs=3) as stat_pool,
            tc.tile_pool(name="rsc", bufs=3) as rsc_pool,
            tc.tile_pool(name="tps", bufs=2, space="PSUM") as tpsum_pool,
            tc.tile_pool(name="zl", bufs=3) as zl_pool,
        ):
            for b in range(NBLK):
                zt = zl_pool.tile([128, D], F32, tag="zload")
                nc.sync.dma_start(zt[:, :], z[b * 128 : (b + 1) * 128, :])
                zp = tpsum_pool.tile([128, 512], F32, tag="tps")
                nc.tensor.transpose(zp[:, :128], zt[:, :], ident[:, :])
                nc.scalar.copy(znT[b][:, :], zp[:, :128])

            ngrp = SLICE // 512  # 12 full groups + one 256 tail
            grps = [512] * ngrp + ([SLICE % 512] if SLICE % 512 else [])
            col = 0
            for gw in grps:
                G = gw // 128
                rl = rload_pool.tile([128, 4, 128], F32, tag="rload")
                nc.sync.dma_start(
                    rl[:, :G, :],
                    ref[col : col + gw, :].rearrange("(g p) d -> p g d", p=128),
                )
                ssq = stat_pool.tile([128, 4], F32, tag="ssq")
                for g in range(G):
                    sq = sq_pool.tile([128, 128], F32, tag="sq")
                    nc.scalar.activation(
                        sq[:, :],
                        rl[:, g, :],
                        mybir.ActivationFunctionType.Square,
                        accum_out=ssq[:, g : g + 1],
                    )
                sn = stat_pool.tile([128, 4], F32, tag="sn")
                nc.scalar.sqrt(sn[:, :G], ssq[:, :G])
                # clamp so zero-padded rows give 0*big = 0, not NaN
                sc = stat_pool.tile([128, 4], F32, tag="sc")
                nc.vector.tensor_scalar_max(sc[:, :G], sn[:, :G], 1e-20)
                rq = stat_pool.tile([128, 4], F32, tag="rq")
                nc.vector.reciprocal(rq[:, :G], sc[:, :G])

                rsc = rsc_pool.tile([128, 4, 128], F32, tag="rsc")
                for g in range(G):
                    nc.vector.tensor_scalar_mul(
                        rsc[:, g, :], rl[:, g, :], rq[:, g : g + 1]
                    )
                tp = tpsum_pool.tile([128, 512], F32, tag="tps")
                for g in range(G):
                    nc.tensor.transpose(
                        tp[:, g * 128 : (g + 1) * 128],
                        rsc[:, g, :],
                        ident[:, :],
                    )
                nc.scalar.copy(rnT_sl[:, col : col + gw], tp[:, :gw])
                col += gw

        # ---- Phase B: per query block, matmul over slice + max8,
        # then local top-16 merge straight to the output (the global
        # 8-way merge happens on host from the per-core outputs) ----
        with tc.tile_pool(name="sp", bufs=2, space="PSUM") as spsum_pool:
            for b in range(NBLK):
                pcol = 0
                for w_i, w in enumerate(WINS):
                    sp = spsum_pool.tile([128, 2048], F32, tag="sp")
                    for h in range(0, w, 512):
                        hw = min(512, w - h)
                        nc.tensor.matmul(
                            sp[:, h : h + hw],
                            znT[b][:, :],
                            rnT_sl[:, pcol + h : pcol + h + hw],
                            start=True,
                            stop=True,
                        )
                    nc.vector.max(
                        cand[b][:, w_i * 8 : (w_i + 1) * 8], sp[:, :w]
                    )
                    pcol += w
                t8a = fin_pool.tile([128, 8], F32, tag="t8a")
                nc.vector.max(t8a[:, :], cand[b][:, :])
                cand2 = fin_pool.tile([128, CANDW], F32, tag="cand2")
                nc.vector.match_replace(
                    cand2[:, :], t8a[:, :], cand[b][:, :], -3.0
                )
                t8b = fin_pool.tile([128, 8], F32, tag="t8b")
                nc.vector.max(t8b[:, :], cand2[:, :])
                nc.sync.dma_start(
                    out16[b * 128 : (b + 1) * 128, 0:8], t8a[:, :]
                )
                nc.sync.dma_start(
                    out16[b * 128 : (b + 1) * 128, 8:16], t8b[:, :]
                )

    nc.compile()
    return nc


def kernel(z, ref, k):
    global LAST_RESULTS
    z_np = np.ascontiguousarray(np.asarray(z, dtype=np.float32))
    ref_np = np.ascontiguousarray(np.asarray(ref, dtype=np.float32))
    kk = int(k)

    if "nc" not in _CACHE:
        _CACHE["nc"] = _build()
    nc = _CACHE["nc"]

    refp = np.zeros((NCORES * SLICE, D), dtype=np.float32)
    refp[:M] = ref_np
    in_maps = [
        {
            "z": z_np,
            "refsl": np.ascontiguousarray(refp[i * SLICE : (i + 1) * SLICE]),
        }
        for i in range(NCORES)
    ]
    res = run_bass_kernel_spmd(nc, in_maps, core_ids=list(range(NCORES)))
    LAST_RESULTS = res
    # each core returns its slice-local top-16 per query; merge on host
    allc = np.concatenate(
        [r["top16"] for r in res.results], axis=1
    )  # [N, 128]
    allc.sort(axis=1)
    s_k = allc[:, -(kk + 1)]  # (k+1)-th largest raw score

    znorm = np.sqrt(np.sum(z_np.astype(np.float32) ** 2, axis=1))  # [N]
    s = s_k / znorm
    return np.sqrt(np.maximum(2.0 - 2.0 * s, 1e-12)).astype(np.float32)


# revision 12
# speedup vs baseline: 2.3578x; 1.4087x over previous
"""KNN k-th-nearest-distance kernel for Trainium2 (8 NeuronCores).

Math: reference computes, per query row n, the k-th smallest of
dist[n,m] = sqrt(max(||zn||^2 + ||rn||^2 - 2 zn.rn, 1e-12)) over m,
with zn/rn the L2-normalized rows of z [2048,128] / ref [50000,128].
Since ||zn||^2 == ||rn||^2 == 1 (to fp32 rounding), dist is a
monotonically *decreasing* function of s = zn.rn, so the k-th smallest
distance corresponds to the k-th *largest* inner product s.

Device work, v4 (ref sharded across cores, queries replicated):
  Phase A (per core, own 1/8 ref slice, zero-padded to 6400 rows):
    normalize rows in fp32 (square-accum on ScalarE, reciprocal on
    VectorE, scale on GpSimd), PE-transpose -> rnT slice [128, 6400]
    fp32 resident in SBUF. 1/norm = 1/max(sqrt(ssq), 1e-20) so padded
    zero rows give sims of exactly 0 (never competitive: real top-11
    sims are ~ +0.37|z| for randn data).
  Phase B: for each of the 16 query blocks, s_raw = z_blk @ rnT_slice
    via fp32 PE matmul into 2048-wide PSUM windows; DVE max8 per
    window -> per-(block, window) top-8 candidates.
  Merge: per block, max8 + match_replace + max8 -> this core's local
    top-16 per query; AllGather the [2048,16] candidate matrices
    (128KB) and reduce 8x16 -> exact global top-16 per query.
Host: divide by |z_n|, dist = sqrt(max(2 - 2*s, 1e-12)), pick column k.
"""

import os
import sys

sys.path.insert(0, "/opt/trn_rl_repo")

from contextlib import ExitStack

import numpy as np


def _install_ntff_hook_shim():
    """The agent image's antenv lacks axon_hooks, so trace=True degrades.
    Recreate the hook module + the ctypes NTFF driver (mirrors
    trn_agent_boot.trn_boot._ntff_profile_via_ctypes)."""
    import contextlib
    import ctypes
    import types

    if "antenv.axon_hooks" in sys.modules:
        return
    so_path = "/opt/axon/libaxon_pjrt.so"
    mod = types.ModuleType("antenv.axon_hooks")
    state = {"hook": None}

    def set_axon_ntff_profile_hook(h):
        state["hook"] = h

    def get_axon_ntff_profile_hook():
        return state["hook"]

    mod.set_axon_ntff_profile_hook = set_axon_ntff_profile_hook
    mod.get_axon_ntff_profile_hook = get_axon_ntff_profile_hook
    sys.modules["antenv.axon_hooks"] = mod

    try:
        lib = ctypes.CDLL(so_path)
        if not hasattr(lib, "axon_start_nrt_profile"):
            return
        lib.axon_start_nrt_profile.argtypes = [
            ctypes.POINTER(ctypes.c_int64),
            ctypes.c_size_t,
        ]
        lib.axon_start_nrt_profile.restype = ctypes.c_int64
        lib.axon_stop_nrt_profile.argtypes = [ctypes.c_char_p]
        lib.axon_stop_nrt_profile.restype = ctypes.c_int64

        @contextlib.contextmanager
        def _hook(output_dir, device_ids):
            import jax

            jax.devices()
            if device_ids:
                ids = (ctypes.c_int64 * len(device_ids))(*device_ids)
                rc = lib.axon_start_nrt_profile(ids, len(device_ids))
            else:
                rc = lib.axon_start_nrt_profile(None, 0)
            if rc != 0:
                raise RuntimeError(f"axon_start_nrt_profile rc={rc}")
            try:
                yield
            finally:
                n = lib.axon_stop_nrt_profile(str(output_dir).encode())
                print(f"ntff profile: {n} file(s) written to {output_dir}")

        state["hook"] = _hook
    except Exception:
        pass


_install_ntff_hook_shim()

import concourse.bacc as bacc
import concourse.bass as bass
import concourse.tile as tile
from concourse import mybir
from concourse.bass_utils import run_bass_kernel_spmd

N, M, D = 2048, 50000, 128
NCORES = 8
NBLK = N // 128  # 16 query blocks of 128
SLICE = 6400  # ref rows prepped per core (core 7 zero-padded)
WINS = (2048, 2048, 2048, 256)  # max8 windows over the 6400 slice cols
NWIN = len(WINS)
CANDW = NWIN * 8  # per-core candidate width per query row (32)

F32 = mybir.dt.float32
BF16 = mybir.dt.bfloat16

_CACHE = {}
LAST_RESULTS = None


def _build():
    nc = bacc.Bacc(
        "TRN2", target_bir_lowering=False, debug=False, num_devices=NCORES
    )
    z_d = nc.dram_tensor("z", [N, D], F32, kind="ExternalInput")
    ref_d = nc.dram_tensor("refsl", [SLICE, D], F32, kind="ExternalInput")
    out_d = nc.dram_tensor("top16", [N, 16], F32, kind="ExternalOutput")
    import ml_dtypes

    ident_d = nc.inline_tensor(
        np.eye(128, dtype=np.float32).astype(ml_dtypes.bfloat16), name="ident"
    )

    z = z_d.ap()
    ref = ref_d.ap()
    out16 = out_d.ap()

    with tile.TileContext(nc) as tc, ExitStack() as ctx:
        const_pool = ctx.enter_context(tc.tile_pool(name="const", bufs=1))
        persist = ctx.enter_context(tc.tile_pool(name="persist", bufs=1))
        cand_pool = ctx.enter_context(tc.tile_pool(name="cand", bufs=1))
        fin_pool = ctx.enter_context(tc.tile_pool(name="fin", bufs=2))

        ident = const_pool.tile([128, 128], BF16, name="ident")
        nc.sync.dma_start(ident[:, :], ident_d.ap()[:, :])

        rnT_sl = persist.tile([128, SLICE], BF16, name="rnT_sl")
        znT = [
            persist.tile([128, 128], BF16, tag=f"znT{b}", name=f"znT{b}")
            for b in range(NBLK)
        ]
        cand = [
            cand_pool.tile([128, CANDW], F32, tag=f"cand{b}", name=f"cand{b}")
            for b in range(NBLK)
        ]

        # ---- Phase A: z transposes + own-slice normalize/transpose ----
        with (
            tc.tile_pool(name="rload", bufs=3) as rload_pool,
            tc.tile_pool(name="sq", bufs=2) as sq_pool,
            tc.tile_pool(name="stat", bufs=3) as stat_pool,
            tc.tile_pool(name="rsc", bufs=3) as rsc_pool,
            tc.tile_pool(name="tps", bufs=2, space="PSUM") as tpsum_pool,
            tc.tile_pool(name="zl", bufs=3) as zl_pool,
        ):
            for b in range(NBLK):
                zt = zl_pool.tile([128, D], F32, tag="zload")
                nc.sync.dma_start(zt[:, :], z[b * 128 : (b + 1) * 128, :])
                ztb = zl_pool.tile([128, D], BF16, tag="zloadb")
                nc.vector.tensor_copy(ztb[:, :], zt[:, :])
                zp = tpsum_pool.tile([128, 512], BF16, tag="tps")
                nc.tensor.transpose(zp[:, :128], ztb[:, :], ident[:, :])
                nc.scalar.copy(znT[b][:, :], zp[:, :128])

            ngrp = SLICE // 512  # 12 full groups + one 256 tail
            grps = [512] * ngrp + ([SLICE % 512] if SLICE % 512 else [])
            col = 0
            for gw in grps:
                G = gw // 128
                rl = rload_pool.tile([128, 4, 128], F32, tag="rload")
                nc.sync.dma_start(
                    rl[:, :G, :],
                    ref[col : col + gw, :].rearrange("(g p) d -> p g d", p=128),
                )
                ssq = stat_pool.tile([128, 4], F32, tag="ssq")
                for g in range(G):
                    sq = sq_pool.tile([128, 128], F32, tag="sq")
                    nc.scalar.activation(
                        sq[:, :],
                        rl[:, g, :],
                        mybir.ActivationFunctionType.Square,
                        accum_out=ssq[:, g : g + 1],
                    )
                sn = stat_pool.tile([128, 4], F32, tag="sn")
                nc.scalar.sqrt(sn[:, :G], ssq[:, :G])
                # clamp so zero-padded rows give 0*big = 0, not NaN
                sc = stat_pool.tile([128, 4], F32, tag="sc")
                nc.vector.tensor_scalar_max(sc[:, :G], sn[:, :G], 1e-20)
                rq = stat_pool.tile([128, 4], F32, tag="rq")
                nc.vector.reciprocal(rq[:, :G], sc[:, :G])

                rsc = rsc_pool.tile([128, 4, 128], BF16, tag="rsc")
                for g in range(G):
                    nc.vector.tensor_scalar_mul(
                        rsc[:, g, :], rl[:, g, :], rq[:, g : g + 1]
                    )
                tp = tpsum_pool.tile([128, 512], BF16, tag="tps")
                for g in range(G):
                    nc.tensor.transpose(
                        tp[:, g * 128 : (g + 1) * 128],
                        rsc[:, g, :],
                        ident[:, :],
                    )
                nc.scalar.copy(rnT_sl[:, col : col + gw], tp[:, :gw])
                col += gw

        # ---- Phase B: per query block, matmul over slice + max8,
        # then local top-16 merge straight to the output (the global
        # 8-way merge happens on host from the per-core outputs) ----
        with tc.tile_pool(name="sp", bufs=2, space="PSUM") as spsum_pool:
            for b in range(NBLK):
                pcol = 0
                for w_i, w in enumerate(WINS):
                    sp = spsum_pool.tile([128, 2048], F32, tag="sp")
                    for h in range(0, w, 512):
                        hw = min(512, w - h)
                        nc.tensor.matmul(
                            sp[:, h : h + hw],
                            znT[b][:, :],
                            rnT_sl[:, pcol + h : pcol + h + hw],
                            start=True,
                            stop=True,
                        )
                    nc.vector.max(
                        cand[b][:, w_i * 8 : (w_i + 1) * 8], sp[:, :w]
                    )
                    pcol += w
                t8a = fin_pool.tile([128, 8], F32, tag="t8a")
                nc.vector.max(t8a[:, :], cand[b][:, :])
                cand2 = fin_pool.tile([128, CANDW], F32, tag="cand2")
                nc.vector.match_replace(
                    cand2[:, :], t8a[:, :], cand[b][:, :], -3.0
                )
                t8b = fin_pool.tile([128, 8], F32, tag="t8b")
                nc.vector.max(t8b[:, :], cand2[:, :])
                nc.sync.dma_start(
                    out16[b * 128 : (b + 1) * 128, 0:8], t8a[:, :]
                )
                nc.sync.dma_start(
                    out16[b * 128 : (b + 1) * 128, 8:16], t8b[:, :]
                )

    nc.compile()
    return nc


def kernel(z, ref, k):
    global LAST_RESULTS
    z_np = np.ascontiguousarray(np.asarray(z, dtype=np.float32))
    ref_np = np.ascontiguousarray(np.asarray(ref, dtype=np.float32))
    kk = int(k)

    if "nc" not in _CACHE:
        _CACHE["nc"] = _build()
    nc = _CACHE["nc"]

    refp = np.zeros((NCORES * SLICE, D), dtype=np.float32)
    refp[:M] = ref_np
    in_maps = [
        {
            "z": z_np,
            "refsl": np.ascontiguousarray(refp[i * SLICE : (i + 1) * SLICE]),
        }
        for i in range(NCORES)
    ]
    res = run_bass_kernel_spmd(nc, in_maps, core_ids=list(range(NCORES)))
    LAST_RESULTS = res
    # each core returns its slice-local top-16 per query; merge on host
    allc = np.concatenate(
        [r["top16"] for r in res.results], axis=1
    )  # [N, 128]
    allc.sort(axis=1)
    s_k = allc[:, -(kk + 1)]  # (k+1)-th largest raw score

    znorm = np.sqrt(np.sum(z_np.astype(np.float32) ** 2, axis=1))  # [N]
    s = s_k / znorm
    return np.sqrt(np.maximum(2.0 - 2.0 * s, 1e-12)).astype(np.float32)
